# revision 19
# baseline (speedup 1.0000x reference)
"""DINO-style loss kernel for Trainium2, SPMD over 8 NeuronCores.

Math (matches the jax reference):
  centroids_c = segment_mean over queue rows with info_label==c; the /count
  cancels under L2-normalize, so centroids_norm = l2norm(segment_SUM).
  pseudo_label[b] = argmax_c batch[b]·centroids_norm[c]
  MAE[b,k] = sqrt(2 - 2*batch[b]·queue[k] + 1e-6)
  loss = mean_b(masked-row-mean) + 2 - mean_b(complement-row-mean)

Key restructuring for the hardware: the per-row masked sums over K factor
through the 100 classes:
  sum_k MAE[b,k]*[label_k==p_b] = G[p_b, b]  where  G = onehot(label).T @ MAE
so the whole [B,K] mask never materializes: one streaming pass over the
queue computes (a) centroid sums + class counts, (b) sim -> MAE, and
(c) G, all as fp8 DoubleRow PE matmuls (2 k-tiles packed per matmul).
A small epilogue picks row p_b via an equality mask against the per-column
max of the class-similarity matrix.

The elementwise bottleneck is sqrt over all B*K sim values. Only the ACT
engine has a sqrt, and only ACT/DVE may read PSUM, so the work is split
three ways per k-tile pair:
  'A' pairs: ACT computes mae = sqrt(psum + 2.000001) directly (the sim
     matmul's queue operand is pre-scaled by -2 so psum = u = -2*sim).
  'P'/'V' pairs: DVE computes v = VA*u + VB (one pass, drains PSUM), then
     Pool ('P') or DVE ('V') squares it: m = v*v = KP - mae_quad(u), a
     completed-square minimax quadratic (max rel err 3.1e-3). Since m is
     KP - mae (the square is convex, mae concave), the host NEGATES the
     one-hot AND queue rows of poly pairs: the G matmul then accumulates
     (-oh)*(KP - mae) = oh*mae - KP*oh, fixed up in the epilogue by
     + KP*cnt_poly per class; the centroid matmul sees (-oh)@(-q) = oh@q.
  Class counts accumulate in two psum columns (col 256: 'A' pairs,
  col 257: negated poly pairs) and are combined in the epilogue.

The one-hot label matrix (padded to 128 columns - DoubleRow requires the
stationary free size to be exactly 128) is precomputed on the host as fp8.

Sharding: data-parallel over B (512 rows/core); queue/labels replicated.
Each core emits [sum_b masked_mean, sum_b complement_mean]; host combines.
"""

import math

import numpy as np
import ml_dtypes

import concourse.bacc as bacc
import concourse.bass as bass
import concourse.mybir as mybir
import concourse.tile as tile
from concourse.bass_utils import run_bass_kernel_spmd

# Problem constants (hardcoded per contract).
B, K, D, C = 4096, 32768, 256, 100
NCORES = 8
BL = B // NCORES          # 512 rows of batch per core
CH = 2048                 # queue rows per DMA chunk
NCH = K // CH             # 16 chunks
TPC = CH // 128           # 16 k-tiles per chunk
NPAIR = K // 256          # 128 k-tile pairs total
PPC = TPC // 2            # 8 pairs per chunk
EPS_SQRT = 1e-6

F32 = mybir.dt.float32
BF16 = mybir.dt.bfloat16
F8 = mybir.dt.float8e4

_CACHE = {}
# test-harness hooks: extra kwargs for run_bass_kernel_spmd (e.g. trace=True)
# and the last BassKernelResults for timing inspection.
_RUN_KWARGS = {}
_LAST_RESULTS = None

G_DELAY = 10   # pairs of software-pipeline distance between sim and G matmul
V2_DELAY = 2   # pairs of delay before the DVE-squaring pass ('V' pairs)

# Minimax quadratic sqrt(u + 2.000001) ~ A2*u^2 + A1*u + A0 on u in
# [-0.95, 0.95]; completed square: KP - (VA*u + VB)^2.
_A2, _A1, _A0 = -0.04824201139694538, 0.3633521599936369, 1.41497367056488
VA = math.sqrt(-_A2)
VB = -_A1 / (2.0 * VA)
KP = _A0 + VB * VB


def _mae_schedule():
    """Per-pair engine assignment: 'A' ACT-sqrt, 'P' DVE-affine+Pool-square,
    'V' DVE-affine+DVE-square. Cost-model balance: ACT 1038, DVE pass1 1192,
    DVE pass2 1127, Pool pass2 2127 ns -> shares 76/37/15 of 128."""
    import os

    ov = os.environ.get("KERNEL_SCHED", "")
    if ov:
        counts = {"A": 0, "P": 0, "V": 0}
        for part in ov.split(","):
            counts[part[0]] = int(part[1:])
    else:
        counts = {"A": 76, "P": 37, "V": 15}
    assert sum(counts.values()) == NPAIR
    acc = dict.fromkeys(counts, 0.0)
    sched = []
    for _ in range(NPAIR):
        for e in acc:
            acc[e] += counts[e]
        pick = max(acc, key=lambda e: acc[e])
        acc[pick] -= NPAIR
        sched.append(pick)
    return sched


_SCHED = _mae_schedule()


def _build_module(dbg=False):
    nc = bacc.Bacc("TRN2", debug=False, target_bir_lowering=False)
    if dbg:
        dbg_cnt_d = nc.dram_tensor("dbg_cnt", [C, 4], F32, kind="ExternalOutput")
        dbg_gt_d = nc.dram_tensor("dbg_gt", [C, BL], F32, kind="ExternalOutput")
        dbg_row_d = nc.dram_tensor("dbg_row", [4, BL], F32, kind="ExternalOutput")
        dbg_pl_d = nc.dram_tensor("dbg_pl", [1, BL], F32, kind="ExternalOutput")

    # Inputs (per-core). Queue pre-scaled by -2 in qt (sim lhsT); raw fp8 in
    # qb (centroid rhs, sign-baked for poly pairs); one-hot labels in oh
    # (sign-baked, padded to 128 cols).
    qt_d = nc.dram_tensor("qt", [NCH, 128, 2, CH], F8, kind="ExternalInput")
    qb_d = nc.dram_tensor("qb", [NCH, 128, TPC, 256], F8, kind="ExternalInput")
    oh_d = nc.dram_tensor("oh", [NCH, 128, TPC, 128], F8, kind="ExternalInput")
    bt_d = nc.dram_tensor("bt", [2, 128, BL], BF16, kind="ExternalInput")
    bt8_d = nc.dram_tensor("bt8", [128, 2, BL], F8, kind="ExternalInput")
    iota_d = nc.dram_tensor("iota", [128, 128], F32, kind="ExternalInput")
    ident_d = nc.dram_tensor("ident", [128, 128], F32, kind="ExternalInput")
    iotac_d = nc.dram_tensor("iotac", [128, 1], F32, kind="ExternalInput")
    out_d = nc.dram_tensor("out", [1, 2], F32, kind="ExternalOutput")

    sched = _SCHED
    firstA = sched.index("A")
    lastA = NPAIR - 1 - sched[::-1].index("A")
    poly_idx = [t for t, e in enumerate(sched) if e != "A"]
    firstP, lastP = poly_idx[0], poly_idx[-1]

    with tile.TileContext(nc) as tc:
        with (
            tc.tile_pool(name="const", bufs=1) as constp,
            tc.tile_pool(name="stream", bufs=3) as streamp,
            tc.tile_pool(name="mae", bufs=14) as maep,
            tc.tile_pool(name="vee", bufs=6) as veep,
            tc.tile_pool(name="epi", bufs=1) as epip,
            tc.tile_pool(name="pacc", bufs=1, space="PSUM") as paccp,
        ):
            # ---- constants / small inputs ----
            bt8_sb = constp.tile([128, 2, BL], F8)
            nc.sync.dma_start(bt8_sb[:], bt8_d[:])
            bt_sb = constp.tile([128, 2, BL], BF16)
            nc.sync.dma_start(bt_sb[:, 0, :], bt_d[0])
            nc.sync.dma_start(bt_sb[:, 1, :], bt_d[1])
            iota_sb = constp.tile([128, 128], F32)
            nc.sync.dma_start(iota_sb[:], iota_d[:])
            identf_sb = constp.tile([128, 128], F32)
            nc.sync.dma_start(identf_sb[:], ident_d[:])
            iotac_sb = constp.tile([128, 1], F32)
            nc.sync.dma_start(iotac_sb[:], iotac_d[:])
            ident_sb = constp.tile([128, 128], BF16)
            nc.vector.tensor_copy(ident_sb[:], identf_sb[:])
            ones8 = constp.tile([128, 2, 1], F8)
            nc.vector.memset(ones8[:], 1.0)
            ones_f = constp.tile([128, 1], F32)
            nc.vector.memset(ones_f[:], 1.0)
            bias2 = constp.tile([128, 1], F32)
            nc.vector.memset(bias2[:], 2.0 + EPS_SQRT)
            ones_rowb = constp.tile([1, 128], BF16)
            nc.vector.memset(ones_rowb[:], 1.0)

            # ---- persistent PSUM accumulators (1 bank each) ----
            # psum_sc: centroid sums cols 0:256; counts col 256 ('A') and
            # col 257 (negated poly). psum_g: G accumulator [128, 512].
            psum_sc = paccp.tile([128, 512], F32)
            psum_g = paccp.tile([128, 512], F32)

            # ---- streaming loop over the queue ----
            pend_g = []  # (oh_tile, 2j, mae_tile, pair_idx)
            pend_v = []  # (v_tile, mae_tile)
            with tc.tile_pool(name="psim", bufs=3, space="PSUM") as psimp:
                for c in range(NCH):
                    qt = streamp.tile([128, 2, CH], F8, tag="qt")
                    qb = streamp.tile([128, TPC, 256], F8, tag="qb")
                    oh = streamp.tile([128, TPC, 128], F8, tag="oh")
                    if c == 0:
                        # fine-grained first chunk so compute starts early
                        q4 = CH // 4
                        for piece in range(4):
                            sl = slice(piece * q4, (piece + 1) * q4)
                            nc.sync.dma_start(qt[:, :, sl], qt_d[c, :, :, sl])
                        for piece in range(2):
                            tsl = slice(piece * (TPC // 2), (piece + 1) * (TPC // 2))
                            nc.sync.dma_start(oh[:, tsl, :], oh_d[c, :, tsl, :])
                            nc.sync.dma_start(qb[:, tsl, :], qb_d[c, :, tsl, :])
                    else:
                        nc.sync.dma_start(qt[:], qt_d[c])
                        nc.sync.dma_start(oh[:], oh_d[c])
                        nc.sync.dma_start(qb[:], qb_d[c])

                    for j in range(PPC):
                        t = c * PPC + j  # pair index
                        eng = sched[t]
                        psim = psimp.tile([128, 2, BL], F32, tag="sim")
                        for i in range(2):
                            n = 2 * j + i
                            # u = -2 * queue[k]·batch[b]
                            nc.tensor.matmul(
                                psim[:, i, :],
                                qt[:, :, n * 128 : (n + 1) * 128],
                                bt8_sb[:],
                                perf_mode=mybir.MatmulPerfMode.DoubleRow,
                            )
                        # centroid sums += onehot.T @ queue  (fp8 DoubleRow;
                        # poly pairs have both oh and qb negated -> correct)
                        nc.tensor.matmul(
                            psum_sc[:, 0:256],
                            oh[:, 2 * j : 2 * j + 2, :],
                            qb[:, 2 * j : 2 * j + 2, :],
                            start=(t == 0),
                            stop=(t == NPAIR - 1),
                            perf_mode=mybir.MatmulPerfMode.DoubleRow,
                        )
                        # class counts += oh.T @ ones (col 256 for 'A',
                        # col 257 for poly pairs, where oh is negated)
                        col = 256 if eng == "A" else 320
                        nc.tensor.matmul(
                            psum_sc[:, col : col + 1],
                            oh[:, 2 * j : 2 * j + 2, :],
                            ones8[:],
                            start=(t == (firstA if eng == "A" else firstP)),
                            stop=(t == (lastA if eng == "A" else lastP)),
                            perf_mode=mybir.MatmulPerfMode.DoubleRow,
                        )
                        mae = maep.tile([128, 2, BL], F8, tag="mae")
                        if eng == "A":
                            # mae = sqrt(u + 2.000001)
                            nc.scalar.activation(
                                mae[:],
                                psim[:],
                                mybir.ActivationFunctionType.Sqrt,
                                bias=bias2[:],
                                scale=1.0,
                            )
                        else:
                            # v = VA*u + VB  (DVE drains PSUM)
                            v = veep.tile([128, 2, BL], F32, tag="v")
                            nc.vector.tensor_scalar(
                                v[:],
                                psim[:],
                                VA,
                                VB,
                                mybir.AluOpType.mult,
                                mybir.AluOpType.add,
                            )
                            if eng == "P":
                                # m = v*v = KP - mae_quad  (Pool, SBUF only)
                                nc.gpsimd.tensor_tensor(
                                    mae[:], v[:], v[:], mybir.AluOpType.mult
                                )
                            else:
                                # DVE squaring, deferred a couple of pairs so
                                # later pass1's aren't head-of-line blocked
                                pend_v.append((t + 1, v, mae))
                        pend_g.append((oh, 2 * j, mae, t))
                        while pend_v and pend_v[0][0] <= t:
                            _, vv, vmae = pend_v.pop(0)
                            nc.vector.tensor_tensor(
                                vmae[:], vv[:], vv[:], mybir.AluOpType.mult
                            )
                        if len(pend_g) > G_DELAY:
                            goh, gn, gmae, gt_i = pend_g.pop(0)
                            nc.tensor.matmul(
                                psum_g[:, :],
                                goh[:, gn : gn + 2, :],
                                gmae[:],
                                start=(gt_i == 0),
                                stop=(gt_i == NPAIR - 1),
                                perf_mode=mybir.MatmulPerfMode.DoubleRow,
                            )
                # drain the pipelines
                for _, vv, vmae in pend_v:
                    nc.vector.tensor_tensor(
                        vmae[:], vv[:], vv[:], mybir.AluOpType.mult
                    )
                pend_v.clear()
                for goh, gn, gmae, gt_i in pend_g:
                    nc.tensor.matmul(
                        psum_g[:, :],
                        goh[:, gn : gn + 2, :],
                        gmae[:],
                        start=(gt_i == 0),
                        stop=(gt_i == NPAIR - 1),
                        perf_mode=mybir.MatmulPerfMode.DoubleRow,
                    )
                pend_g.clear()

            # ---- epilogue ----
            pepip_cm = tc.tile_pool(name="pepi", bufs=1, space="PSUM")
            pepip = pepip_cm.__enter__()
            # class counts: col256 ('A') minus col257 (negated poly counts)
            colA = epip.tile([C, 1], F32)
            nc.vector.tensor_copy(colA[:], psum_sc[0:C, 256:257])
            colP = epip.tile([C, 1], F32)
            nc.vector.tensor_copy(colP[:], psum_sc[0:C, 320:321])
            counts_col = epip.tile([C, 1], F32)
            nc.vector.tensor_tensor(
                counts_col[:], colA[:], colP[:], mybir.AluOpType.subtract
            )
            # G fixup: gt = psum_g + KP * cnt_poly = psum_g - KP * colP
            kpcnt = epip.tile([C, 1], F32)
            nc.vector.tensor_scalar(
                kpcnt[:], colP[:], -KP, None, mybir.AluOpType.mult
            )
            gt_sb = epip.tile([C, BL], F32)
            nc.vector.tensor_scalar(
                gt_sb[:], psum_g[0:C, :], kpcnt[:], None, mybir.AluOpType.add
            )
            # centroid norms: sq[c] = sum_d sums^2 (ACT Square w/ accum)
            sc_sq = epip.tile([C, 256], F32)
            sq = epip.tile([C, 1], F32)
            nc.scalar.activation(
                sc_sq[:],
                psum_sc[0:C, 0:256],
                mybir.ActivationFunctionType.Square,
                accum_out=sq[:],
            )
            normc = epip.tile([C, 1], F32)
            nc.scalar.activation(normc[:], sq[:], mybir.ActivationFunctionType.Sqrt)
            nc.vector.tensor_scalar(
                normc[:], normc[:], 1e-12, None, mybir.AluOpType.max
            )
            rnorm = epip.tile([C, 1], F32)
            nc.vector.reciprocal(rnorm[:], normc[:])
            # cnorm rows scaled; bf16 for the class-sim matmul
            cnorm = epip.tile([C, 256], BF16)
            nc.vector.tensor_scalar(
                cnorm[:],
                psum_sc[0:C, 0:256],
                rnorm[:],
                None,
                mybir.AluOpType.mult,
            )
            # cnormT [128d, 2, C] via PE transpose (bf16)
            cnormT = epip.tile([128, 2, C], BF16)
            for h in range(2):
                p_tp = pepip.tile([128, C], BF16, tag="tpa")
                nc.tensor.transpose(
                    p_tp[:], cnorm[:, h * 128 : (h + 1) * 128], ident_sb[0:C, 0:C]
                )
                nc.vector.tensor_copy(cnormT[:, h, :], p_tp[:])

            # simcT[b, c] = batch[b]·cnorm[c], per 128-row b-block; then
            # per-block argmax over classes via eq-trick.
            epia_cm = tc.tile_pool(name="epia", bufs=4)
            epia = epia_cm.__enter__()
            plrow_sb = epip.tile([1, BL], BF16)
            for bt_i in range(4):
                p_sc = pepip.tile([128, C], F32, tag="simb")
                for h in range(2):
                    nc.tensor.matmul(
                        p_sc[:],
                        bt_sb[:, h, bt_i * 128 : (bt_i + 1) * 128],
                        cnormT[:, h, :],
                        start=(h == 0),
                        stop=(h == 1),
                    )
                scb = epia.tile([128, C], F32, tag="scb")
                nc.vector.tensor_copy(scb[:], p_sc[:])
                mx = epia.tile([128, 1], F32, tag="mx")
                nc.vector.tensor_reduce(
                    mx[:], scb[:], mybir.AxisListType.X, mybir.AluOpType.max
                )
                eq = epia.tile([128, C], F32, tag="eq")
                nc.vector.tensor_scalar(
                    eq[:], scb[:], mx[:], None, mybir.AluOpType.is_equal
                )
                eqi = epia.tile([128, C], F32, tag="eqi")
                nc.vector.tensor_tensor(
                    eqi[:], eq[:], iota_sb[:, :C], mybir.AluOpType.mult
                )
                plc = epia.tile([128, 1], F32, tag="plc")
                nc.vector.tensor_reduce(
                    plc[:], eqi[:], mybir.AxisListType.X, mybir.AluOpType.max
                )
                p_plr = pepip.tile([1, 128], F32, tag="plra")
                nc.tensor.transpose(p_plr[:], plc[:], identf_sb[:, :])
                nc.vector.tensor_copy(
                    plrow_sb[0:1, bt_i * 128 : (bt_i + 1) * 128], p_plr[:]
                )
            epia_cm.__exit__(None, None, None)
            # broadcast pseudo-label row to C partitions via K=1 matmul (bf16:
            # label values <= 99 are exact)
            p_plb = pepip.tile([C, BL], F32, tag="plb")
            nc.tensor.matmul(p_plb[:], ones_rowb[0:1, 0:C], plrow_sb[:])
            # P[c,b] = (plabel[b] == c)
            pmask = epip.tile([C, BL], F32)
            nc.vector.tensor_scalar(
                pmask[:], p_plb[:], iotac_sb[0:C, :], None, mybir.AluOpType.is_equal
            )
            masked = epip.tile([C, BL], F32)
            nc.vector.tensor_tensor(
                masked[:], pmask[:], gt_sb[:], mybir.AluOpType.mult
            )
            cntsel = epip.tile([C, BL], F32)
            nc.vector.tensor_scalar(
                cntsel[:], pmask[:], counts_col[:], None, mybir.AluOpType.mult
            )
            # column sums over the 100 classes via ones-matmuls (fp32)
            r_mask = pepip.tile([1, BL], F32, tag="rs1")
            nc.tensor.matmul(r_mask[:], ones_f[0:C, :], masked[:])
            rm_sb = epip.tile([1, BL], F32)
            nc.vector.tensor_copy(rm_sb[:], r_mask[:])
            r_cnt = pepip.tile([1, BL], F32, tag="rs2")
            nc.tensor.matmul(r_cnt[:], ones_f[0:C, :], cntsel[:])
            # rs1 rotates: safe because rm_sb already holds the masked sums
            r_tot = pepip.tile([1, BL], F32, tag="rs1")
            nc.tensor.matmul(r_tot[:], ones_f[0:C, :], gt_sb[:])
            # per-row terms. cnt + 1e-6 and (K - cnt) + 1e-6 equal cnt and
            # K - cnt exactly under fp32 rounding (counts are O(300)), and
            # the reference rounds identically, so the eps adds are elided.
            rec1 = epip.tile([1, BL], F32)
            nc.vector.reciprocal(rec1[:], r_cnt[:])
            min_t = epip.tile([1, BL], F32)
            nc.vector.tensor_tensor(min_t[:], rm_sb[:], rec1[:], mybir.AluOpType.mult)
            d2 = epip.tile([1, BL], F32)
            nc.vector.tensor_scalar(
                d2[:],
                r_cnt[:],
                -1.0,
                float(K),
                mybir.AluOpType.mult,
                mybir.AluOpType.add,
            )
            rec2 = epip.tile([1, BL], F32)
            nc.vector.reciprocal(rec2[:], d2[:])
            diff = epip.tile([1, BL], F32)
            nc.vector.tensor_tensor(
                diff[:], r_tot[:], rm_sb[:], mybir.AluOpType.subtract
            )
            int_t = epip.tile([1, BL], F32)
            nc.vector.tensor_tensor(int_t[:], diff[:], rec2[:], mybir.AluOpType.mult)
            if dbg:
                dbg_cnt = epip.tile([C, 4], F32)
                nc.vector.tensor_copy(dbg_cnt[:, 0:1], colA[:])
                nc.vector.tensor_copy(dbg_cnt[:, 1:2], colP[:])
                nc.vector.tensor_copy(dbg_cnt[:, 2:3], counts_col[:])
                nc.vector.tensor_copy(dbg_cnt[:, 3:4], kpcnt[:])
                nc.sync.dma_start(dbg_cnt_d[:], dbg_cnt[:])
                nc.sync.dma_start(dbg_gt_d[:], gt_sb[:])
                dbg_rc = epip.tile([1, BL], F32)
                nc.vector.tensor_copy(dbg_rc[:], r_cnt[:])
                dbg_rt = epip.tile([1, BL], F32)
                nc.vector.tensor_copy(dbg_rt[:], r_tot[:])
                nc.sync.dma_start(dbg_row_d[0:1, :], rm_sb[:])
                nc.sync.dma_start(dbg_row_d[1:2, :], dbg_rc[:])
                nc.sync.dma_start(dbg_row_d[2:3, :], dbg_rt[:])
                nc.sync.dma_start(dbg_row_d[3:4, :], min_t[:])
                dbg_pl = epip.tile([1, BL], F32)
                nc.vector.tensor_copy(dbg_pl[:], plrow_sb[:])
                nc.sync.dma_start(dbg_pl_d[:], dbg_pl[:])
            out_sb = epip.tile([1, 2], F32)
            nc.vector.tensor_reduce(
                out_sb[0:1, 0:1], min_t[:], mybir.AxisListType.X, mybir.AluOpType.add
            )
            nc.vector.tensor_reduce(
                out_sb[0:1, 1:2], int_t[:], mybir.AxisListType.X, mybir.AluOpType.add
            )
            nc.sync.dma_start(out_d[:], out_sb[:])
            pepip_cm.__exit__(None, None, None)

    nc.finalize()
    return nc


def _prep_shared(queue_emb_copy, info_label):
    q = np.asarray(queue_emb_copy, np.float32)
    lab = np.asarray(info_label).astype(np.int64)
    # per-row sign: -1 for rows belonging to poly ('P'/'V') pairs
    sgn = np.ones((K, 1), np.float32)
    for t, e in enumerate(_SCHED):
        if e != "A":
            sgn[t * 256 : (t + 1) * 256] = -1.0
    # qt[c, d_lo, h, j] = fp8(-2 * queue[c*CH + j, 128h + d_lo])  (DoubleRow
    # lhsT for the sim matmul; NOT sign-baked)
    qT8 = np.ascontiguousarray((-2.0 * q).astype(ml_dtypes.float8_e4m3).T)  # [256, K]
    qt = np.ascontiguousarray(qT8.reshape(2, 128, NCH, CH).transpose(2, 1, 0, 3))
    # qb[c, p, n, d] = fp8(sgn * queue[c*CH + n*128 + p, d])
    qb = np.ascontiguousarray(
        (sgn * q)
        .astype(ml_dtypes.float8_e4m3)
        .reshape(NCH, TPC, 128, 256)
        .transpose(0, 2, 1, 3)
    )
    # oh[c, p, n, cls] = fp8(sgn * (label[c*CH + n*128 + p] == cls)), padded
    # to 128 cols (cols >= C always zero; DoubleRow needs M=128)
    oh_full = (
        (lab[:, None] == np.arange(128, dtype=np.int64)[None, :]).astype(np.float32)
        * sgn
    ).astype(ml_dtypes.float8_e4m3)
    oh = np.ascontiguousarray(oh_full.reshape(NCH, TPC, 128, 128).transpose(0, 2, 1, 3))
    iota = np.broadcast_to(np.arange(128, dtype=np.float32)[None, :], (128, 128)).copy()
    ident = np.eye(128, dtype=np.float32)
    iotac = np.arange(128, dtype=np.float32)[:, None].copy()
    return qt, qb, oh, iota, ident, iotac


def make_in_maps(batch_feature, queue_emb_copy, info_label):
    bf = np.asarray(batch_feature, np.float32)
    assert bf.shape == (B, D)
    qt, qb, oh, iota, ident, iotac = _prep_shared(queue_emb_copy, info_label)
    in_maps = []
    for core in range(NCORES):
        bsh = bf[core * BL : (core + 1) * BL]  # [BL, D]
        bt = np.ascontiguousarray(bsh.T.astype(ml_dtypes.bfloat16).reshape(2, 128, BL))
        bt8 = np.ascontiguousarray(
            bsh.T.astype(ml_dtypes.float8_e4m3).reshape(2, 128, BL).transpose(1, 0, 2)
        )
        in_maps.append(
            {
                "qt": qt,
                "qb": qb,
                "oh": oh,
                "bt": bt,
                "bt8": bt8,
                "iota": iota,
                "ident": ident,
                "iotac": iotac,
            }
        )
    return in_maps


def kernel(batch_feature, queue_emb_copy, info_label, num_classes):
    assert int(num_classes) == C

    key = "nc"
    if key not in _CACHE:
        _CACHE[key] = _build_module()
    nc = _CACHE[key]

    in_maps = make_in_maps(batch_feature, queue_emb_copy, info_label)

    global _LAST_RESULTS
    res = run_bass_kernel_spmd(nc, in_maps, core_ids=list(range(NCORES)), **_RUN_KWARGS)
    _LAST_RESULTS = res
    acc = np.zeros(2, np.float64)
    for r in res.results:
        acc += np.asarray(r["out"], np.float64).reshape(2)
    loss = np.float32(acc[0] / B + 2.0 - acc[1] / B)
    return np.asarray(loss, dtype=np.float32)
